# revision 19
# baseline (speedup 1.0000x reference)
"""Trainium2 Bass kernel for nn_MultiHeadAttention (fused QKV + RMS-norm +
RoPE + masked softmax attention + out-proj), tensor-parallel over heads
across 8 NeuronCores.

Contract: kernel(**inputs) takes FULL inputs, returns FULL output.
Self-contained: imports only numpy + the concourse framework.

v3: PE-warmup block, 3-way DMA-trigger spread (sync/scalar/gpsimd),
qk/v weight-stream split with paired tile-0 x, pqkfo psum bufs=3,
stats+sums merged into one PSUM bank (rows 0/32 ssq, 64/96 sums) with
batched Ln/Exp and back-to-back 1-row matmuls (PE column-group overlap),
swap-trick rope (6 DVE ops/chain), exp shift removed, out-proj evac
split DVE/ACT/gpsimd with out-DMA triggers on sync.
"""
import numpy as np

import concourse.bacc as bacc
import concourse.mybir as mybir
import concourse.tile as tile
from concourse import bass_utils

# Pin every scalar activation to the one table set that holds all functions
# this kernel uses (Exp, Ln, Square, Copy, Identity) so the table-load
# placement pass never needs a mid-kernel reload. Other sets are presented
# as empty; dict order (= act_func_set_id) is preserved.
_PINNED_SET = "natural_log_exp_and_others"
_orig_get_act_tables = bacc.get_activation_tables


def _pinned_act_tables(arch):
    t = _orig_get_act_tables(arch)
    return {name: (funcs if name == _PINNED_SET else set())
            for name, funcs in t.items()}


bacc.get_activation_tables = _pinned_act_tables

N_HEAD = 16
ROPE_BASE = 100000.0
RMS_EPS = 1e-5
L = 2048
D = 2048
HD = 128          # head dim
N_CORES = 8
HPD = N_HEAD // N_CORES   # heads per device = 2
QT = 512          # q tile (attention + l-tile width)
NQT = L // QT     # 4
NKB = L // 128    # 16 k-blocks
NEG = -1.0e9
SCALE = 1.0 / np.sqrt(HD)

F32 = mybir.dt.float32
F16 = mybir.dt.float16

_prog_cache: dict = {}


def _classify_mask(bm: np.ndarray):
    """Per (q-tile, k-block) classification of the boolean mask.

    plan[qt][kb] = ('full',) | ('skip',) | ('part', bias_idx, bstart, bw,
    zstart, zw): add biases[bias_idx][:, :bw] to score cols
    [bstart:bstart+bw], cols [zstart:zstart+zw] are fully masked. biases is
    [n, 128, 512] fp32 (col-sliced bias subtiles, zero-padded)."""
    plan = []
    uniq = {}
    biases = []
    for qt in range(NQT):
        row = []
        for kb in range(NKB):
            reg = bm[qt * QT:(qt + 1) * QT, kb * 128:(kb + 1) * 128]
            if reg.all():
                row.append(("full",))
                continue
            if not reg.any():
                row.append(("skip",))
                continue
            regT = reg.T  # [128 k, 512 q]
            col_all = regT.all(axis=0)       # fully open columns
            col_none = (~regT).all(axis=0)   # fully masked columns
            mixed = ~(col_all | col_none)
            mix_idx = np.nonzero(mixed)[0]
            none_idx = np.nonzero(col_none)[0]
            # contiguity of ranges (true for causal masks)
            ok = True
            bs = bw = zs = zw = 0
            if mix_idx.size:
                bs, be = int(mix_idx[0]), int(mix_idx[-1]) + 1
                bw = be - bs
                ok &= bool(mixed[bs:be].all())
            if none_idx.size:
                zs, ze = int(none_idx[0]), int(none_idx[-1]) + 1
                zw = ze - zs
                ok &= bool(col_none[zs:ze].all())
                ok &= not (bw and not (ze <= bs or be <= zs))
            if not ok:
                # fallback: full-width bias
                bs, bw, zs, zw = 0, QT, 0, 0
            if bw:
                bias = np.zeros((128, QT), np.float32)
                bias[:, 0:bw] = np.where(regT[:, bs:bs + bw], np.float32(0),
                                         np.float32(NEG))
                key = (bw, bias.tobytes())
                if key not in uniq:
                    uniq[key] = len(biases)
                    biases.append(bias)
                bi = uniq[key]
            else:
                bi = 0
            row.append(("part", bi, bs, bw, zs, zw))
        plan.append(tuple(row))
    if not biases:
        biases.append(np.zeros((128, QT), np.float32))
    return tuple(plan), np.stack(biases)


def _restrict(ent, first):
    """Start column for compute on this block (0 = full width).

    Only left-contiguous fully-masked column runs can be skipped, and the
    first block of an accumulation group must cover all columns."""
    if first or ent[0] != "part":
        return 0
    _, bi, bs, bw, zs, zw = ent
    if zw and zs == 0 and (bw == 0 or bs >= zw):
        return zw
    return 0


def _build_program(plan, n_bias):
    nc = bacc.Bacc("TRN2", target_bir_lowering=False, debug=False)

    # ---- DRAM I/O ----
    # all x streams pair-packed: row-block j holds d-chunks 2j,2j+1 side by
    # side -> 2/4KB DMA lines
    xT0p_d = nc.dram_tensor("xT0p", [D // 2, 2 * QT], F16, kind="ExternalInput")
    xT1p_d = nc.dram_tensor("xT1p", [D // 2, 2 * QT], F16,
                            kind="ExternalInput")
    xT23p_d = nc.dram_tensor("xT23p", [D // 2, 4 * QT], F16,
                             kind="ExternalInput")
    # weights split into qk / v streams (pair-packed like x)
    wqkT_d = nc.dram_tensor("wqkT", [D // 2, 2 * 4 * HD], F16,
                            kind="ExternalInput")
    wvT_d = nc.dram_tensor("wvT", [D // 2, 2 * 2 * HD], F16,
                           kind="ExternalInput")
    woutT_d = nc.dram_tensor("woutT", [2 * HD, D], F16, kind="ExternalInput")
    # rope tables pre-duplicated for the swap-trick rope:
    # cos2 = [cosT; cosT], sin2m = [-sinT; +sinT]  (both [128, L])
    cos2_d = nc.dram_tensor("cos2", [128, L], F16, kind="ExternalInput")
    sin2m_d = nc.dram_tensor("sin2m", [128, L], F16, kind="ExternalInput")
    ones128_d = nc.dram_tensor("ones128", [128, 1], F16, kind="ExternalInput")
    bias_d = nc.dram_tensor("biasT", [n_bias, 128, QT], F32, kind="ExternalInput")
    out_d = nc.dram_tensor("out", [L, D], F16, kind="ExternalOutput")

    Exp = mybir.ActivationFunctionType.Exp
    Ln = mybir.ActivationFunctionType.Ln
    Square = mybir.ActivationFunctionType.Square

    with nc.allow_low_precision(reason="fp16 operands, fp32 accumulation"), \
         tile.TileContext(nc) as tc:
        with (
            tc.tile_pool(name="const", bufs=1) as cpool,
            tc.tile_pool(name="xt", bufs=25) as xtpool,
            tc.tile_pool(name="act", bufs=1) as apool,
            tc.tile_pool(name="wrk", bufs=1) as wpool,
            tc.tile_pool(name="ps", bufs=1, space="PSUM") as ps,
        ):
            # ---- all input DMA triggers on sync, in priority order (the
            # HWDGE in-flight window is global, extra trigger queues only
            # pollute compute queues with throttle waits) ----
            def dma_in(dst, src):
                nc.sync.dma_start(dst, src)

            # ---- PE warm-up: ~12 dummy matmuls (~5us cold) pre-warm the
            # HAM clock gate while the first x/w slices stream in ----
            warm = cpool.tile([128, QT], F16, name="warm", tag="warm")
            nc.vector.memset(warm[:], 0.25)
            for w in range(12):
                wps = ps.tile([128, QT], F32, name=f"warm{w}", tag="sT",
                              bufs=2)
                nc.tensor.matmul(wps[:], warm[:, 0:128], warm[:],
                                 start=True, stop=True)

            # ---- constants + input streams (priority order) ----
            ones128 = cpool.tile([128, 1], F16, name="ones128", tag="ones128")
            dma_in(ones128[:], ones128_d.ap())
            cos2 = cpool.tile([128, L], F16, name="cos2", tag="cos2")
            dma_in(cos2[:], cos2_d.ap())
            sin2m = cpool.tile([128, L], F16, name="sin2m", tag="sin2m")
            dma_in(sin2m[:], sin2m_d.ap())
            epsc = cpool.tile([128, 1], F32, name="epsc", tag="epsc")
            nc.vector.memset(epsc[:], RMS_EPS)

            # tile-0 x in 128KB pieces (low first-piece latency) interleaved
            # with qk and v weight streams
            wqk_all = cpool.tile([128, 8 * 2 * QT], F16, name="wqk_all",
                                 tag="wqk_all")
            wv_all = cpool.tile([128, 8 * 512], F16, name="wv_all",
                                tag="wv_all")
            xts0 = []
            for i in range(16):
                xt = xtpool.tile([128, QT], F16, name="xt0", tag="xt0",
                                 bufs=16)
                dma_in(xt[:], xT0p_d.ap()[(i // 2) * 128:(i // 2 + 1) * 128,
                                          (i % 2) * QT:(i % 2 + 1) * QT])
                xts0.append(xt)
                if i % 2 == 1:
                    j = i // 2
                    dma_in(wqk_all[:, j * 1024:(j + 1) * 1024],
                           wqkT_d.ap()[j * 128:(j + 1) * 128, :])
                    dma_in(wv_all[:, j * 512:(j + 1) * 512],
                           wvT_d.ap()[j * 128:(j + 1) * 128, :])
            # mask biases (needed by qt0 attention)
            btiles = []
            for b in range(n_bias):
                bt = cpool.tile([128, QT], F32, name=f"bias{b}", tag=f"bias{b}")
                dma_in(bt[:], bias_d.ap()[b])
                btiles.append(bt)
            # tile-1 x
            xts1 = []
            for j in range(8):
                xt = xtpool.tile([128, 2 * QT], F16, name="xt1", tag="xt1",
                                 bufs=8)
                dma_in(xt[:], xT1p_d.ap()[j * 128:(j + 1) * 128, :])
                xts1.append(xt)
            # out-proj weights (needed from attention qt1)
            wout_all = cpool.tile([128, 2 * D], F16, name="wout_all",
                                  tag="wout_all")
            for h in range(2):
                dma_in(wout_all[:, h * D:(h + 1) * D],
                       woutT_d.ap()[h * 128:(h + 1) * 128, :])
            # tiles 2+3 x
            xts23 = []
            for j in range(8):
                xt = xtpool.tile([128, 4 * QT], F16, name="xt23", tag="xt23",
                                 bufs=8)
                dma_in(xt[:], xT23p_d.ap()[j * 128:(j + 1) * 128, :])
                xts23.append(xt)

            # ---- persistent activations (fp16) ----
            ktr = [apool.tile([128, L], F16, name=f"ktr{h}", tag=f"ktr{h}")
                   for h in range(HPD)]
            aot = [apool.tile([128, L], F16, name=f"aot{h}", tag=f"aot{h}")
                   for h in range(HPD)]
            vnat = [apool.tile([128, 2 * HD], F16, name=f"vnat{lb}",
                               tag=f"vnat{lb}") for lb in range(16)]

            # ---------- out-projection unit (emitted as attention filler) ----
            # out-DMA triggers lag one unit behind so the sync queue's
            # trigger wait is ~zero by issue time (ob_sb already evacuated)
            pending_dmas = []

            def flush_dmas(alternate=False):
                k = 0
                while pending_dmas:
                    dst, src = pending_dmas.pop(0)
                    eng = nc.gpsimd if (alternate and k % 2) else nc.sync
                    eng.dma_start(dst, src)
                    k += 1

            def make_outproj_units(lt, split=False):
                units = []
                for j in range(4):
                    lb = 4 * lt + j
                    for jp in range(2):  # jt pairs (0,1) and (2,3)
                        def unit(lb=lb, jp=jp):
                            flush_dmas()
                            ob_sb = wpool.tile([128, 2 * QT], F16,
                                               name="ob_sb", tag="ob_sb",
                                               bufs=6)
                            for u in range(2):
                                jt = 2 * jp + u
                                fo = ps.tile([128, QT], F32,
                                             name=f"fo{lb}_{jt}",
                                             tag="pqkfo", bufs=3)
                                for h in range(2):
                                    nc.tensor.matmul(
                                        fo[:],
                                        aot[h][:, lb * 128:(lb + 1) * 128],
                                        wout_all[:, h * D + jt * QT:
                                                 h * D + (jt + 1) * QT],
                                        start=(h == 0), stop=(h == 1),
                                    )
                                dst = ob_sb[:, u * QT:(u + 1) * QT]
                                if u == 0:
                                    nc.vector.tensor_copy(out=dst, in_=fo[:])
                                else:
                                    nc.scalar.copy(dst, fo[:])
                                if split:
                                    pending_dmas.append((
                                        out_d.ap()[lb * 128:(lb + 1) * 128,
                                                   jt * QT:(jt + 1) * QT],
                                        dst,
                                    ))
                            if not split:
                                pending_dmas.append((
                                    out_d.ap()[lb * 128:(lb + 1) * 128,
                                               jp * 2 * QT:(jp + 1) * 2 * QT],
                                    ob_sb[:],
                                ))
                        units.append(unit)
                return units

            pending_units = []

            for lt in range(NQT):
                ls = lt * QT
                # ---------- QKV projection for this l-tile ----------
                if lt == 0:
                    xts = [(xts0[i], 0) for i in range(16)]
                elif lt == 1:
                    xts = [(xts1[i // 2], (i % 2) * QT) for i in range(16)]
                else:
                    xts = [(xts23[i // 2], (i % 2) * 2 * QT + (lt - 2) * QT)
                           for i in range(16)]

                # stats+sums PSUM bank: rows 0/32 = ssq (q-pair then k-pair),
                # rows 64/96 = softmax sums (h0/h1)
                stats = ps.tile([128, QT], F32, name=f"stats{lt}",
                                tag="stats", bufs=1)

                # q/k chains: ob = 0,1 -> q heads 0,1 ; ob = 2,3 -> k heads
                # 0,1; q-pair first so q-rope hides under the k chains
                pp = {}
                qtr_lt = []
                for t in (0, 1):
                    obs = (2 * t, 2 * t + 1)
                    for ob in obs:
                        pp[ob] = ps.tile([128, QT], F32,
                                         name=f"pqk{lt}_{ob}", tag="pqkfo",
                                         bufs=3)
                    if lt == 0:
                        # DMA-inflow-bound: i-outer over the pair
                        for i in range(16):
                            xt, xb = xts[i]
                            for ob in obs:
                                nc.tensor.matmul(
                                    pp[ob][:],
                                    wqk_all[:, (i // 2) * 1024 +
                                            (i % 2) * 512 + ob * 128:
                                            (i // 2) * 1024 +
                                            (i % 2) * 512 + (ob + 1) * 128],
                                    xt[:, xb:xb + QT],
                                    start=(i == 0), stop=(i == 15),
                                )
                    else:
                        for ob in obs:
                            for i in range(16):
                                xt, xb = xts[i]
                                nc.tensor.matmul(
                                    pp[ob][:],
                                    wqk_all[:, (i // 2) * 1024 +
                                            (i % 2) * 512 + ob * 128:
                                            (i // 2) * 1024 +
                                            (i % 2) * 512 + (ob + 1) * 128],
                                    xt[:, xb:xb + QT],
                                    start=(i == 0), stop=(i == 15),
                                )

                    # pair stats: squares on ACT, then two 1-row matmuls
                    # into stats rows 0/32, one batched Ln + Exp over rows
                    # 0..32 (rows 1..31 are garbage, never read)
                    sq = {}
                    for h, ob in enumerate(obs):
                        sqt = wpool.tile([128, QT], F16, name="sq", tag="sq",
                                         bufs=3)
                        nc.scalar.activation(sqt[:], pp[ob][:], Square)
                        sq[ob] = sqt
                    nc.tensor.matmul(stats[0:1, :], ones128[:],
                                     sq[obs[0]][:], start=True, stop=True)
                    nc.tensor.matmul(stats[32:33, :], ones128[:],
                                     sq[obs[1]][:], start=True, stop=True)
                    # s4 = exp(-0.5*ln(ssq/HD + eps)) = 1/sqrt(mean+eps)
                    lg = wpool.tile([33, QT], F32, name="lg", tag="lg",
                                    bufs=2)
                    nc.scalar.activation(lg[:], stats[0:33, :], Ln,
                                         bias=epsc[0:33, :], scale=1.0 / HD)
                    s4b = wpool.tile([33, QT], F16, name="s4", tag="s4",
                                     bufs=2)
                    nc.scalar.activation(s4b[:], lg[:], Exp, scale=-0.5)
                    # h1's scale row copied to a base-0 tile for
                    # partition_broadcast
                    s4h1 = wpool.tile([1, QT], F16, name="s4h1", tag="s4h1",
                                      bufs=2)
                    nc.vector.tensor_copy(out=s4h1[:], in_=s4b[32:33, :])
                    srows = {obs[0]: s4b[0:1, :], obs[1]: s4h1[0:1, :]}

                    # rms-scale + swap-trick rope for the pair:
                    # out[0:64]   = odd*cos - even*sin
                    # out[64:128] = odd*sin + even*cos
                    # via A = scaled*[cos;cos], B = swap(scaled)*[-sin;sin]
                    for h, ob in enumerate(obs):
                        s2 = wpool.tile([128, QT], F16, name="s2", tag="s2",
                                        bufs=3)
                        nc.gpsimd.partition_broadcast(s2[:], srows[ob])
                        # scale reads the raw projection straight from PSUM
                        # (saves a PSUM->SBUF cast per chain)
                        scaled = wpool.tile([128, QT], F16, name="scaled",
                                            tag="scaled", bufs=2)
                        nc.vector.tensor_mul(out=scaled[:], in0=pp[ob][:],
                                             in1=s2[:])
                        sw = wpool.tile([128, QT], F16, name="sw", tag="sw",
                                        bufs=2)
                        nc.vector.tensor_copy(out=sw[0:64, :],
                                              in_=scaled[64:128, :])
                        nc.vector.tensor_copy(out=sw[64:128, :],
                                              in_=scaled[0:64, :])
                        ta = wpool.tile([128, QT], F16, name="ta", tag="ta",
                                        bufs=1)
                        nc.vector.tensor_mul(out=ta[:], in0=scaled[:],
                                             in1=cos2[:, ls:ls + QT])
                        tb = wpool.tile([128, QT], F16, name="tb", tag="tb",
                                        bufs=1)
                        nc.vector.tensor_mul(out=tb[:], in0=sw[:],
                                             in1=sin2m[:, ls:ls + QT])
                        if t == 0:
                            dst = wpool.tile([128, QT], F16, name="qtr",
                                             tag="qtr", bufs=4)
                            nc.vector.tensor_add(out=dst[:], in0=ta[:],
                                                 in1=tb[:])
                            qtr_lt.append(dst)
                        else:
                            nc.vector.tensor_add(
                                out=ktr[h][:, ls:ls + QT],
                                in0=ta[:], in1=tb[:])

                # v projection: natural layout [l, d] per 128-l block
                for j in range(4):
                    lb = 4 * lt + j
                    vp = ps.tile([128, QT], F32, name=f"vp{lb}", tag="pqkfo",
                                 bufs=3)
                    for i in range(16):
                        xt, xb = xts[i]
                        nc.tensor.matmul(
                            vp[:, 0:256],
                            xt[:, xb + j * 128:xb + (j + 1) * 128],
                            wv_all[:, (i // 2) * 512 + (i % 2) * 256:
                                   (i // 2) * 512 + (i % 2 + 1) * 256],
                            start=(i == 0), stop=(i == 15),
                        )
                    nc.vector.tensor_copy(out=vnat[lb][:], in_=vp[:, 0:256])

                # ---------- attention for q-tile qt = lt ----------
                qt = lt
                alive = [kb for kb in range(NKB) if plan[qt][kb][0] != "skip"]
                nblk = len(alive)
                nunits = len(pending_units)
                oT = [ps.tile([128, QT], F32, name=f"oT{qt}_{h}", tag="oT",
                              bufs=2) for h in range(2)]
                emitted = 0
                # h1's sums matmul lags one block so the pair
                # [sums_h1(n-1), sums_h0(n)] is issued adjacently with both
                # pt inputs long-ready -> the two 1-row matmuls overlap in
                # distinct PE column groups (rows 0/64)
                lag1 = None  # (pt_tile, zr, start, stop) for h1's prev block
                for n, kb in enumerate(alive):
                    ent = plan[qt][kb]
                    zr = _restrict(ent, n == 0)
                    sts = []
                    for h in range(2):
                        st = ps.tile([128, QT], F32, name=f"sT{qt}_{h}_{n}",
                                     tag="sT", bufs=2)
                        nc.tensor.matmul(
                            st[:, zr:],
                            ktr[h][:, kb * 128:(kb + 1) * 128],
                            qtr_lt[h][:, zr:],
                            start=True, stop=True,
                        )
                        sts.append(st)
                    if ent[0] == "part":
                        _, bi, bs, bw, zs, zw = ent
                        for h in range(2):
                            if bw:
                                nc.vector.tensor_add(
                                    out=sts[h][:, bs:bs + bw],
                                    in0=sts[h][:, bs:bs + bw],
                                    in1=btiles[bi][:, 0:bw])
                            if zw and zr == 0:
                                nc.vector.tensor_scalar_add(
                                    out=sts[h][:, zs:zs + zw],
                                    in0=sts[h][:, zs:zs + zw],
                                    scalar1=NEG)
                    pts = []
                    for h in range(2):
                        pt = wpool.tile([128, QT], F16, name="pt", tag="pt",
                                        bufs=6)
                        nc.scalar.activation(pt[:, zr:], sts[h][:, zr:], Exp,
                                             scale=SCALE)
                        pts.append(pt)
                    if lag1 is not None:
                        lpt, lzr, lstart, lstop = lag1
                        nc.tensor.matmul(
                            stats[64:65, lzr:], ones128[:], lpt[:, lzr:],
                            start=lstart, stop=lstop)
                    nc.tensor.matmul(
                        stats[0:1, zr:], ones128[:], pts[0][:, zr:],
                        start=(n == 0), stop=(n == nblk - 1))
                    lag1 = (pts[1], zr, n == 0, n == nblk - 1)
                    for h in range(2):
                        nc.tensor.matmul(
                            oT[h][:, zr:],
                            vnat[kb][:, h * HD:(h + 1) * HD],
                            pts[h][:, zr:],
                            start=(n == 0), stop=(n == nblk - 1),
                        )
                    # interleave out-proj filler from the previous l-tile;
                    # on the last l-tile hold 2 units back to cover the final
                    # normalize latency after the block loop
                    cap = nunits - 2 if lt == NQT - 1 else nunits
                    want = min((nunits * (n + 1)) // nblk, cap)
                    while emitted < want:
                        pending_units[emitted]()
                        emitted += 1
                while emitted < nunits:
                    pending_units[emitted]()
                    emitted += 1
                # flush the lagged h1 sums for the final block
                if lag1 is not None:
                    lpt, lzr, lstart, lstop = lag1
                    nc.tensor.matmul(
                        stats[64:65, lzr:], ones128[:], lpt[:, lzr:],
                        start=lstart, stop=lstop)

                # normalize: aot[h][:, qt] = oT * (1/sums) bcast over
                # partitions; per-head recips so h0's broadcast starts while
                # h1's recip runs (shortens the critical path on the last
                # tile, whose out-proj waits on this)
                rr = []
                for h in range(2):
                    if h == 0:
                        src = stats[0:1, :]
                    else:
                        # DVE table ops need base-partition-0 input; copy
                        # row 64 down first
                        s1 = wpool.tile([1, QT], F32, name="s1", tag="s1",
                                        bufs=2)
                        nc.vector.tensor_copy(out=s1[:], in_=stats[64:65, :])
                        src = s1[0:1, :]
                    r = wpool.tile([1, QT], F32, name=f"r{h}", tag=f"r{h}",
                                   bufs=2)
                    nc.vector.reciprocal_approx_fast(out=r[:], in_=src)
                    rr.append(r)
                for h in range(2):
                    rb = wpool.tile([128, QT], F32, name="rb", tag="rb",
                                    bufs=2)
                    nc.gpsimd.partition_broadcast(rb[:], rr[h][0:1, :])
                    nc.vector.tensor_mul(
                        out=aot[h][:, qt * QT:(qt + 1) * QT],
                        in0=oT[h][:], in1=rb[:],
                    )

                pending_units = make_outproj_units(lt, split=(lt == NQT - 1))

            # last l-tile's out-projection; drain DMAs on two queues
            for unit in pending_units:
                unit()
            flush_dmas(alternate=True)

    nc.finalize()
    return nc


def _rope_perm(h):
    """Row order within one head's 128 q/k features: odd indices then even."""
    base = h * HD
    return np.concatenate([np.arange(1, HD, 2), np.arange(0, HD, 2)]) + base


def _pack16(a):
    """[2048, w] -> [1024, 2w]: row-block j holds d-chunks 2j, 2j+1 side by
    side (bigger DMA lines)."""
    w = a.shape[1]
    r = a.reshape(16, 128, w)
    return np.ascontiguousarray(
        np.concatenate([r[0::2], r[1::2]], axis=2).reshape(1024, 2 * w))


def _host_prep(x, W_qkv, W_out):
    xT = np.ascontiguousarray(x[0].T.astype(np.float16))
    xT0p = _pack16(xT[:, 0:QT])
    xT1p = _pack16(xT[:, QT:2 * QT])
    xT23p = _pack16(xT[:, 2 * QT:4 * QT])
    inv_freq = 1.0 / (ROPE_BASE ** (np.arange(0, HD, 2, dtype=np.float64) / HD))
    ang = np.arange(L, dtype=np.float64)[:, None] * inv_freq[None, :]
    cosT = np.cos(ang).T
    sinT = np.sin(ang).T
    cos2 = np.ascontiguousarray(
        np.vstack([cosT, cosT]).astype(np.float16))
    sin2m = np.ascontiguousarray(
        np.vstack([-sinT, sinT]).astype(np.float16))
    ones128 = np.ones((128, 1), np.float16)

    per_core = []
    for d in range(N_CORES):
        h0 = HPD * d
        rows_q = np.concatenate([_rope_perm(h0), _rope_perm(h0 + 1)])
        rows = np.concatenate(
            [rows_q, D + rows_q,
             2 * D + np.arange(h0 * HD, (h0 + 2) * HD)]
        )
        wl = W_qkv[rows, :]                                     # [768, 2048]
        wqkvT = wl.T.astype(np.float16)                         # [2048, 768]
        wqkT = _pack16(wqkvT[:, 0:512])                         # [1024, 1024]
        wvT = _pack16(wqkvT[:, 512:768])                        # [1024, 512]
        woutT = np.ascontiguousarray(
            W_out[:, h0 * HD:(h0 + 2) * HD].T.astype(np.float16)
        )                                                       # [256, 2048]
        per_core.append((wqkT, wvT, woutT))
    return xT0p, xT1p, xT23p, cos2, sin2m, ones128, per_core


def kernel(x, W_qkv, W_out, block_mask):
    x = np.asarray(x, dtype=np.float32)
    W_qkv = np.asarray(W_qkv, dtype=np.float32)
    W_out = np.asarray(W_out, dtype=np.float32)
    bm = np.asarray(block_mask).astype(bool)

    plan, biases = _classify_mask(bm)
    key = (plan, biases.shape[0])
    if key not in _prog_cache:
        _prog_cache[key] = _build_program(plan, biases.shape[0])
    nc = _prog_cache[key]

    xT0p, xT1p, xT23p, cos2, sin2m, ones128, per_core = _host_prep(
        x, W_qkv, W_out)
    in_maps = []
    for d in range(N_CORES):
        wqkT, wvT, woutT = per_core[d]
        in_maps.append({
            "xT0p": xT0p, "xT1p": xT1p, "xT23p": xT23p,
            "wqkT": wqkT, "wvT": wvT, "woutT": woutT,
            "cos2": cos2, "sin2m": sin2m,
            "ones128": ones128, "biasT": biases,
        })
    res = bass_utils.run_bass_kernel_spmd(nc, in_maps, list(range(N_CORES)))
    acc = np.zeros((L, D), np.float64)
    for r in res.results:
        acc += r["out"].astype(np.float64)
    return acc.astype(np.float32)[None, :, :]


# revision 23
# speedup vs baseline: 1.0500x; 1.0500x over previous
"""Trainium2 Bass kernel for nn_MultiHeadAttention (fused QKV + RMS-norm +
RoPE + masked softmax attention + out-proj), tensor-parallel over heads
across 8 NeuronCores.

Contract: kernel(**inputs) takes FULL inputs, returns FULL output.
Self-contained: imports only numpy + the concourse framework.

v3: PE-warmup block, 3-way DMA-trigger spread (sync/scalar/gpsimd),
qk/v weight-stream split with paired tile-0 x, pqkfo psum bufs=3,
stats+sums merged into one PSUM bank (rows 0/32 ssq, 64/96 sums) with
batched Ln/Exp and back-to-back 1-row matmuls (PE column-group overlap),
swap-trick rope (6 DVE ops/chain), exp shift removed, out-proj evac
split DVE/ACT/gpsimd with out-DMA triggers on sync.
"""
import numpy as np

import concourse.bacc as bacc
import concourse.mybir as mybir
import concourse.tile as tile
from concourse import bass_utils

# Pin every scalar activation to the one table set that holds all functions
# this kernel uses (Exp, Ln, Square, Copy, Identity) so the table-load
# placement pass never needs a mid-kernel reload. Other sets are presented
# as empty; dict order (= act_func_set_id) is preserved.
_PINNED_SET = "natural_log_exp_and_others"
_orig_get_act_tables = bacc.get_activation_tables


def _pinned_act_tables(arch):
    t = _orig_get_act_tables(arch)
    return {name: (funcs if name == _PINNED_SET else set())
            for name, funcs in t.items()}


bacc.get_activation_tables = _pinned_act_tables

N_HEAD = 16
ROPE_BASE = 100000.0
RMS_EPS = 1e-5
L = 2048
D = 2048
HD = 128          # head dim
N_CORES = 8
HPD = N_HEAD // N_CORES   # heads per device = 2
QT = 512          # q tile (attention + l-tile width)
NQT = L // QT     # 4
NKB = L // 128    # 16 k-blocks
NEG = -1.0e9
SCALE = 1.0 / np.sqrt(HD)

F32 = mybir.dt.float32
F16 = mybir.dt.float16

_prog_cache: dict = {}


def _classify_mask(bm: np.ndarray):
    """Per (q-tile, k-block) classification of the boolean mask.

    plan[qt][kb] = ('full',) | ('skip',) | ('part', bias_idx, bstart, bw,
    zstart, zw): add biases[bias_idx][:, :bw] to score cols
    [bstart:bstart+bw], cols [zstart:zstart+zw] are fully masked. biases is
    [n, 128, 512] fp32 (col-sliced bias subtiles, zero-padded)."""
    plan = []
    uniq = {}
    biases = []
    for qt in range(NQT):
        row = []
        for kb in range(NKB):
            reg = bm[qt * QT:(qt + 1) * QT, kb * 128:(kb + 1) * 128]
            if reg.all():
                row.append(("full",))
                continue
            if not reg.any():
                row.append(("skip",))
                continue
            regT = reg.T  # [128 k, 512 q]
            col_all = regT.all(axis=0)       # fully open columns
            col_none = (~regT).all(axis=0)   # fully masked columns
            mixed = ~(col_all | col_none)
            mix_idx = np.nonzero(mixed)[0]
            none_idx = np.nonzero(col_none)[0]
            # contiguity of ranges (true for causal masks)
            ok = True
            bs = bw = zs = zw = 0
            if mix_idx.size:
                bs, be = int(mix_idx[0]), int(mix_idx[-1]) + 1
                bw = be - bs
                ok &= bool(mixed[bs:be].all())
            if none_idx.size:
                zs, ze = int(none_idx[0]), int(none_idx[-1]) + 1
                zw = ze - zs
                ok &= bool(col_none[zs:ze].all())
                ok &= not (bw and not (ze <= bs or be <= zs))
            if not ok:
                # fallback: full-width bias
                bs, bw, zs, zw = 0, QT, 0, 0
            if bw:
                bias = np.zeros((128, QT), np.float32)
                bias[:, 0:bw] = np.where(regT[:, bs:bs + bw], np.float32(0),
                                         np.float32(NEG))
                key = (bw, bias.tobytes())
                if key not in uniq:
                    uniq[key] = len(biases)
                    biases.append(bias)
                bi = uniq[key]
            else:
                bi = 0
            row.append(("part", bi, bs, bw, zs, zw))
        plan.append(tuple(row))
    if not biases:
        biases.append(np.zeros((128, QT), np.float32))
    return tuple(plan), np.stack(biases)


def _restrict(ent, first):
    """Start column for compute on this block (0 = full width).

    Only left-contiguous fully-masked column runs can be skipped, and the
    first block of an accumulation group must cover all columns."""
    if first or ent[0] != "part":
        return 0
    _, bi, bs, bw, zs, zw = ent
    if zw and zs == 0 and (bw == 0 or bs >= zw):
        return zw
    return 0


def _build_program(plan, n_bias):
    nc = bacc.Bacc("TRN2", target_bir_lowering=False, debug=False)

    # ---- DRAM I/O ----
    # all x streams pair-packed: row-block j holds d-chunks 2j,2j+1 side by
    # side -> 2/4KB DMA lines
    xT0p_d = nc.dram_tensor("xT0p", [D // 2, 2 * QT], F16, kind="ExternalInput")
    xT1p_d = nc.dram_tensor("xT1p", [D // 2, 2 * QT], F16,
                            kind="ExternalInput")
    xT23p_d = nc.dram_tensor("xT23p", [D // 2, 4 * QT], F16,
                             kind="ExternalInput")
    # weights split into qk / v streams (pair-packed like x)
    wqkT_d = nc.dram_tensor("wqkT", [D // 2, 2 * 4 * HD], F16,
                            kind="ExternalInput")
    wvT_d = nc.dram_tensor("wvT", [D // 2, 2 * 2 * HD], F16,
                           kind="ExternalInput")
    woutT_d = nc.dram_tensor("woutT", [2 * HD, D], F16, kind="ExternalInput")
    # rope tables pre-duplicated for the swap-trick rope:
    # cos2 = [cosT; cosT], sin2m = [-sinT; +sinT]  (both [128, L])
    cos2_d = nc.dram_tensor("cos2", [128, L], F16, kind="ExternalInput")
    sin2m_d = nc.dram_tensor("sin2m", [128, L], F16, kind="ExternalInput")
    ones128_d = nc.dram_tensor("ones128", [128, 1], F16, kind="ExternalInput")
    bias_d = nc.dram_tensor("biasT", [n_bias, 128, QT], F32, kind="ExternalInput")
    out_d = nc.dram_tensor("out", [L, D], F16, kind="ExternalOutput")

    Exp = mybir.ActivationFunctionType.Exp
    Ln = mybir.ActivationFunctionType.Ln
    Square = mybir.ActivationFunctionType.Square

    with nc.allow_low_precision(reason="fp16 operands, fp32 accumulation"), \
         tile.TileContext(nc) as tc:
        with (
            tc.tile_pool(name="const", bufs=1) as cpool,
            tc.tile_pool(name="xt", bufs=25) as xtpool,
            tc.tile_pool(name="act", bufs=1) as apool,
            tc.tile_pool(name="wrk", bufs=1) as wpool,
            tc.tile_pool(name="ps", bufs=1, space="PSUM") as ps,
        ):
            # ---- all input DMA triggers on sync, in priority order (the
            # HWDGE in-flight window is global, extra trigger queues only
            # pollute compute queues with throttle waits) ----
            def dma_in(dst, src):
                nc.sync.dma_start(dst, src)

            # ---- PE warm-up: ~12 dummy matmuls (~5us cold) pre-warm the
            # HAM clock gate while the first x/w slices stream in ----
            warm = cpool.tile([128, QT], F16, name="warm", tag="warm")
            nc.vector.memset(warm[:], 0.25)
            for w in range(12):
                wps = ps.tile([128, QT], F32, name=f"warm{w}", tag="sT",
                              bufs=2)
                nc.tensor.matmul(wps[:], warm[:, 0:128], warm[:],
                                 start=True, stop=True)

            # ---- constants + input streams (priority order) ----
            ones128 = cpool.tile([128, 1], F16, name="ones128", tag="ones128")
            dma_in(ones128[:], ones128_d.ap())
            cos2 = cpool.tile([128, L], F16, name="cos2", tag="cos2")
            dma_in(cos2[:], cos2_d.ap())
            sin2m = cpool.tile([128, L], F16, name="sin2m", tag="sin2m")
            dma_in(sin2m[:], sin2m_d.ap())
            epsc = cpool.tile([128, 1], F32, name="epsc", tag="epsc")
            nc.vector.memset(epsc[:], RMS_EPS)

            # tile-0 x (paired) interleaved with qk weight pairs
            wqk_all = cpool.tile([128, 8 * 2 * QT], F16, name="wqk_all",
                                 tag="wqk_all")
            xts0 = []
            for j in range(8):
                xt = xtpool.tile([128, 2 * QT], F16, name="xt0", tag="xt0",
                                 bufs=8)
                dma_in(xt[:], xT0p_d.ap()[j * 128:(j + 1) * 128, :])
                xts0.append(xt)
                dma_in(wqk_all[:, j * 1024:(j + 1) * 1024],
                       wqkT_d.ap()[j * 128:(j + 1) * 128, :])
            # v weights (needed after the first qk chains)
            wv_all = cpool.tile([128, 8 * 512], F16, name="wv_all",
                                tag="wv_all")
            for j in range(8):
                dma_in(wv_all[:, j * 512:(j + 1) * 512],
                       wvT_d.ap()[j * 128:(j + 1) * 128, :])
            # mask biases (needed by qt0 attention)
            btiles = []
            for b in range(n_bias):
                bt = cpool.tile([128, QT], F32, name=f"bias{b}", tag=f"bias{b}")
                dma_in(bt[:], bias_d.ap()[b])
                btiles.append(bt)
            # tile-1 x
            xts1 = []
            for j in range(8):
                xt = xtpool.tile([128, 2 * QT], F16, name="xt1", tag="xt1",
                                 bufs=8)
                dma_in(xt[:], xT1p_d.ap()[j * 128:(j + 1) * 128, :])
                xts1.append(xt)
            # out-proj weights (needed from attention qt1)
            wout_all = cpool.tile([128, 2 * D], F16, name="wout_all",
                                  tag="wout_all")
            for h in range(2):
                dma_in(wout_all[:, h * D:(h + 1) * D],
                       woutT_d.ap()[h * 128:(h + 1) * 128, :])
            # tiles 2+3 x
            xts23 = []
            for j in range(8):
                xt = xtpool.tile([128, 4 * QT], F16, name="xt23", tag="xt23",
                                 bufs=8)
                dma_in(xt[:], xT23p_d.ap()[j * 128:(j + 1) * 128, :])
                xts23.append(xt)

            # ---- persistent activations (fp16) ----
            ktr = [apool.tile([128, L], F16, name=f"ktr{h}", tag=f"ktr{h}")
                   for h in range(HPD)]
            aot = [apool.tile([128, L], F16, name=f"aot{h}", tag=f"aot{h}")
                   for h in range(HPD)]
            vnat = [apool.tile([128, 2 * HD], F16, name=f"vnat{lb}",
                               tag=f"vnat{lb}") for lb in range(16)]

            # ---------- out-projection unit (emitted as attention filler) ----
            # out-DMA triggers lag one unit behind so the sync queue's
            # trigger wait is ~zero by issue time (ob_sb already evacuated)
            pending_dmas = []

            def flush_dmas(alternate=False):
                k = 0
                while pending_dmas:
                    dst, src = pending_dmas.pop(0)
                    eng = nc.gpsimd if (alternate and k % 2) else nc.sync
                    eng.dma_start(dst, src)
                    k += 1

            def make_outproj_units(lt, tail=False):
                units = []
                for idx, (j, jp) in enumerate(
                        (j, jp) for j in range(4) for jp in range(2)):
                    lb = 4 * lt + j
                    # the final units' 256KB out-DMAs are the kernel drain:
                    # split them so pieces ride parallel DMA lanes
                    pieces = 1
                    if tail and idx == 6:
                        pieces = 2
                    elif tail and idx == 7:
                        pieces = 4

                    def unit(lb=lb, jp=jp, pieces=pieces, tail=tail):
                        flush_dmas(alternate=tail)
                        ob_sb = wpool.tile([128, 2 * QT], F16,
                                           name="ob_sb", tag="ob_sb",
                                           bufs=6)
                        for u in range(2):
                            jt = 2 * jp + u
                            fo = ps.tile([128, QT], F32,
                                         name=f"fo{lb}_{jt}",
                                         tag="pqkfo", bufs=3)
                            for h in range(2):
                                nc.tensor.matmul(
                                    fo[:],
                                    aot[h][:, lb * 128:(lb + 1) * 128],
                                    wout_all[:, h * D + jt * QT:
                                             h * D + (jt + 1) * QT],
                                    start=(h == 0), stop=(h == 1),
                                )
                            dst = ob_sb[:, u * QT:(u + 1) * QT]
                            if u == 0:
                                nc.vector.tensor_copy(out=dst, in_=fo[:])
                            else:
                                nc.scalar.copy(dst, fo[:])
                        pw = (2 * QT) // pieces
                        for p in range(pieces):
                            pending_dmas.append((
                                out_d.ap()[lb * 128:(lb + 1) * 128,
                                           jp * 2 * QT + p * pw:
                                           jp * 2 * QT + (p + 1) * pw],
                                ob_sb[:, p * pw:(p + 1) * pw],
                            ))
                    units.append(unit)
                return units

            pending_units = []

            for lt in range(NQT):
                ls = lt * QT
                # ---------- QKV projection for this l-tile ----------
                if lt == 0:
                    xts = [(xts0[i // 2], (i % 2) * QT) for i in range(16)]
                elif lt == 1:
                    xts = [(xts1[i // 2], (i % 2) * QT) for i in range(16)]
                else:
                    xts = [(xts23[i // 2], (i % 2) * 2 * QT + (lt - 2) * QT)
                           for i in range(16)]

                # stats+sums PSUM bank: rows 0/32 = ssq (q-pair then k-pair),
                # rows 64/96 = softmax sums (h0/h1)
                stats = ps.tile([128, QT], F32, name=f"stats{lt}",
                                tag="stats", bufs=1)

                # q/k chains: ob = 0,1 -> q heads 0,1 ; ob = 2,3 -> k heads
                # 0,1; q-pair first so q-rope hides under the k chains
                pp = {}
                qtr_lt = []
                for t in (0, 1):
                    obs = (2 * t, 2 * t + 1)
                    for ob in obs:
                        pp[ob] = ps.tile([128, QT], F32,
                                         name=f"pqk{lt}_{ob}", tag="pqkfo",
                                         bufs=3)
                    if lt == 0:
                        # DMA-inflow-bound: i-outer over the pair
                        for i in range(16):
                            xt, xb = xts[i]
                            for ob in obs:
                                nc.tensor.matmul(
                                    pp[ob][:],
                                    wqk_all[:, (i // 2) * 1024 +
                                            (i % 2) * 512 + ob * 128:
                                            (i // 2) * 1024 +
                                            (i % 2) * 512 + (ob + 1) * 128],
                                    xt[:, xb:xb + QT],
                                    start=(i == 0), stop=(i == 15),
                                )
                    else:
                        for ob in obs:
                            for i in range(16):
                                xt, xb = xts[i]
                                nc.tensor.matmul(
                                    pp[ob][:],
                                    wqk_all[:, (i // 2) * 1024 +
                                            (i % 2) * 512 + ob * 128:
                                            (i // 2) * 1024 +
                                            (i % 2) * 512 + (ob + 1) * 128],
                                    xt[:, xb:xb + QT],
                                    start=(i == 0), stop=(i == 15),
                                )

                    # pair stats: squares on ACT, then two 1-row matmuls
                    # into stats rows 0/32, one batched Ln + Exp over rows
                    # 0..32 (rows 1..31 are garbage, never read)
                    sq = {}
                    for h, ob in enumerate(obs):
                        sqt = wpool.tile([128, QT], F16, name="sq", tag="sq",
                                         bufs=3)
                        nc.scalar.activation(sqt[:], pp[ob][:], Square)
                        sq[ob] = sqt
                    nc.tensor.matmul(stats[0:1, :], ones128[:],
                                     sq[obs[0]][:], start=True, stop=True)
                    nc.tensor.matmul(stats[32:33, :], ones128[:],
                                     sq[obs[1]][:], start=True, stop=True)
                    # s4 = exp(-0.5*ln(ssq/HD + eps)) = 1/sqrt(mean+eps)
                    lg = wpool.tile([33, QT], F32, name="lg", tag="lg",
                                    bufs=2)
                    nc.scalar.activation(lg[:], stats[0:33, :], Ln,
                                         bias=epsc[0:33, :], scale=1.0 / HD)
                    s4b = wpool.tile([33, QT], F16, name="s4", tag="s4",
                                     bufs=2)
                    nc.scalar.activation(s4b[:], lg[:], Exp, scale=-0.5)
                    # h1's scale row copied to a base-0 tile for
                    # partition_broadcast
                    s4h1 = wpool.tile([1, QT], F16, name="s4h1", tag="s4h1",
                                      bufs=2)
                    nc.vector.tensor_copy(out=s4h1[:], in_=s4b[32:33, :])
                    srows = {obs[0]: s4b[0:1, :], obs[1]: s4h1[0:1, :]}

                    # rms-scale + swap-trick rope for the pair:
                    # out[0:64]   = odd*cos - even*sin
                    # out[64:128] = odd*sin + even*cos
                    # via A = scaled*[cos;cos], B = swap(scaled)*[-sin;sin]
                    for h, ob in enumerate(obs):
                        s2 = wpool.tile([128, QT], F16, name="s2", tag="s2",
                                        bufs=3)
                        nc.gpsimd.partition_broadcast(s2[:], srows[ob])
                        # scale reads the raw projection straight from PSUM
                        # (saves a PSUM->SBUF cast per chain)
                        scaled = wpool.tile([128, QT], F16, name="scaled",
                                            tag="scaled", bufs=2)
                        nc.vector.tensor_mul(out=scaled[:], in0=pp[ob][:],
                                             in1=s2[:])
                        sw = wpool.tile([128, QT], F16, name="sw", tag="sw",
                                        bufs=2)
                        nc.vector.tensor_copy(out=sw[0:64, :],
                                              in_=scaled[64:128, :])
                        nc.vector.tensor_copy(out=sw[64:128, :],
                                              in_=scaled[0:64, :])
                        ta = wpool.tile([128, QT], F16, name="ta", tag="ta",
                                        bufs=1)
                        nc.vector.tensor_mul(out=ta[:], in0=scaled[:],
                                             in1=cos2[:, ls:ls + QT])
                        tb = wpool.tile([128, QT], F16, name="tb", tag="tb",
                                        bufs=1)
                        nc.vector.tensor_mul(out=tb[:], in0=sw[:],
                                             in1=sin2m[:, ls:ls + QT])
                        if t == 0:
                            dst = wpool.tile([128, QT], F16, name="qtr",
                                             tag="qtr", bufs=4)
                            nc.vector.tensor_add(out=dst[:], in0=ta[:],
                                                 in1=tb[:])
                            qtr_lt.append(dst)
                        else:
                            nc.vector.tensor_add(
                                out=ktr[h][:, ls:ls + QT],
                                in0=ta[:], in1=tb[:])

                # v projection: natural layout [l, d] per 128-l block
                for j in range(4):
                    lb = 4 * lt + j
                    vp = ps.tile([128, QT], F32, name=f"vp{lb}", tag="pqkfo",
                                 bufs=3)
                    for i in range(16):
                        xt, xb = xts[i]
                        nc.tensor.matmul(
                            vp[:, 0:256],
                            xt[:, xb + j * 128:xb + (j + 1) * 128],
                            wv_all[:, (i // 2) * 512 + (i % 2) * 256:
                                   (i // 2) * 512 + (i % 2 + 1) * 256],
                            start=(i == 0), stop=(i == 15),
                        )
                    nc.vector.tensor_copy(out=vnat[lb][:], in_=vp[:, 0:256])

                # ---------- attention for q-tile qt = lt ----------
                qt = lt
                alive = [kb for kb in range(NKB) if plan[qt][kb][0] != "skip"]
                nblk = len(alive)
                nunits = len(pending_units)
                oT = [ps.tile([128, QT], F32, name=f"oT{qt}_{h}", tag="oT",
                              bufs=2) for h in range(2)]
                emitted = 0
                # h1's sums matmul lags one block so the pair
                # [sums_h1(n-1), sums_h0(n)] is issued adjacently with both
                # pt inputs long-ready -> the two 1-row matmuls overlap in
                # distinct PE column groups (rows 0/64)
                lag1 = None  # (pt_tile, zr, start, stop) for h1's prev block
                for n, kb in enumerate(alive):
                    ent = plan[qt][kb]
                    zr = _restrict(ent, n == 0)
                    sts = []
                    for h in range(2):
                        st = ps.tile([128, QT], F32, name=f"sT{qt}_{h}_{n}",
                                     tag="sT", bufs=2)
                        nc.tensor.matmul(
                            st[:, zr:],
                            ktr[h][:, kb * 128:(kb + 1) * 128],
                            qtr_lt[h][:, zr:],
                            start=True, stop=True,
                        )
                        sts.append(st)
                    if ent[0] == "part":
                        _, bi, bs, bw, zs, zw = ent
                        for h in range(2):
                            if bw:
                                nc.vector.tensor_add(
                                    out=sts[h][:, bs:bs + bw],
                                    in0=sts[h][:, bs:bs + bw],
                                    in1=btiles[bi][:, 0:bw])
                            if zw and zr == 0:
                                nc.vector.tensor_scalar_add(
                                    out=sts[h][:, zs:zs + zw],
                                    in0=sts[h][:, zs:zs + zw],
                                    scalar1=NEG)
                    pts = []
                    for h in range(2):
                        pt = wpool.tile([128, QT], F16, name="pt", tag="pt",
                                        bufs=6)
                        nc.scalar.activation(pt[:, zr:], sts[h][:, zr:], Exp,
                                             scale=SCALE)
                        pts.append(pt)
                    if lag1 is not None:
                        lpt, lzr, lstart, lstop = lag1
                        nc.tensor.matmul(
                            stats[64:65, lzr:], ones128[:], lpt[:, lzr:],
                            start=lstart, stop=lstop)
                    nc.tensor.matmul(
                        stats[0:1, zr:], ones128[:], pts[0][:, zr:],
                        start=(n == 0), stop=(n == nblk - 1))
                    lag1 = (pts[1], zr, n == 0, n == nblk - 1)
                    for h in range(2):
                        nc.tensor.matmul(
                            oT[h][:, zr:],
                            vnat[kb][:, h * HD:(h + 1) * HD],
                            pts[h][:, zr:],
                            start=(n == 0), stop=(n == nblk - 1),
                        )
                    # interleave out-proj filler from the previous l-tile;
                    # on the last l-tile hold 2 units back to cover the final
                    # normalize latency after the block loop
                    cap = nunits - 2 if lt == NQT - 1 else nunits
                    want = min((nunits * (n + 1)) // nblk, cap)
                    while emitted < want:
                        pending_units[emitted]()
                        emitted += 1
                while emitted < nunits:
                    pending_units[emitted]()
                    emitted += 1
                # flush the lagged h1 sums for the final block
                if lag1 is not None:
                    lpt, lzr, lstart, lstop = lag1
                    nc.tensor.matmul(
                        stats[64:65, lzr:], ones128[:], lpt[:, lzr:],
                        start=lstart, stop=lstop)

                # normalize: aot[h][:, qt] = oT * (1/sums) bcast over
                # partitions; per-head recips so h0's broadcast starts while
                # h1's recip runs (shortens the critical path on the last
                # tile, whose out-proj waits on this)
                # batched reciprocal over stats rows 0..64 (base-0 input —
                # DVE table ops misread base-64 APs), then h1's row copied
                # down for its broadcast
                rinvb = wpool.tile([65, QT], F32, name="rinvb", tag="rinvb",
                                   bufs=2)
                nc.vector.reciprocal_approx_fast(out=rinvb[:],
                                                 in_=stats[0:65, :])
                r1 = wpool.tile([1, QT], F32, name="r1", tag="r1", bufs=2)
                nc.vector.tensor_copy(out=r1[:], in_=rinvb[64:65, :])
                for h in range(2):
                    rb = wpool.tile([128, QT], F32, name="rb", tag="rb",
                                    bufs=2)
                    nc.gpsimd.partition_broadcast(
                        rb[:], rinvb[0:1, :] if h == 0 else r1[0:1, :])
                    nc.vector.tensor_mul(
                        out=aot[h][:, qt * QT:(qt + 1) * QT],
                        in0=oT[h][:], in1=rb[:],
                    )

                pending_units = make_outproj_units(lt, tail=(lt == NQT - 1))

            # last l-tile's out-projection; drain DMAs on two queues
            for unit in pending_units:
                unit()
            flush_dmas(alternate=True)

    nc.finalize()
    return nc


def _rope_perm(h):
    """Row order within one head's 128 q/k features: odd indices then even."""
    base = h * HD
    return np.concatenate([np.arange(1, HD, 2), np.arange(0, HD, 2)]) + base


def _pack16(a):
    """[2048, w] -> [1024, 2w]: row-block j holds d-chunks 2j, 2j+1 side by
    side (bigger DMA lines)."""
    w = a.shape[1]
    r = a.reshape(16, 128, w)
    return np.ascontiguousarray(
        np.concatenate([r[0::2], r[1::2]], axis=2).reshape(1024, 2 * w))


def _host_prep(x, W_qkv, W_out):
    xT = np.ascontiguousarray(x[0].T.astype(np.float16))
    xT0p = _pack16(xT[:, 0:QT])
    xT1p = _pack16(xT[:, QT:2 * QT])
    xT23p = _pack16(xT[:, 2 * QT:4 * QT])
    inv_freq = 1.0 / (ROPE_BASE ** (np.arange(0, HD, 2, dtype=np.float64) / HD))
    ang = np.arange(L, dtype=np.float64)[:, None] * inv_freq[None, :]
    cosT = np.cos(ang).T
    sinT = np.sin(ang).T
    cos2 = np.ascontiguousarray(
        np.vstack([cosT, cosT]).astype(np.float16))
    sin2m = np.ascontiguousarray(
        np.vstack([-sinT, sinT]).astype(np.float16))
    ones128 = np.ones((128, 1), np.float16)

    per_core = []
    for d in range(N_CORES):
        h0 = HPD * d
        rows_q = np.concatenate([_rope_perm(h0), _rope_perm(h0 + 1)])
        rows = np.concatenate(
            [rows_q, D + rows_q,
             2 * D + np.arange(h0 * HD, (h0 + 2) * HD)]
        )
        wl = W_qkv[rows, :]                                     # [768, 2048]
        wqkvT = wl.T.astype(np.float16)                         # [2048, 768]
        wqkT = _pack16(wqkvT[:, 0:512])                         # [1024, 1024]
        wvT = _pack16(wqkvT[:, 512:768])                        # [1024, 512]
        woutT = np.ascontiguousarray(
            W_out[:, h0 * HD:(h0 + 2) * HD].T.astype(np.float16)
        )                                                       # [256, 2048]
        per_core.append((wqkT, wvT, woutT))
    return xT0p, xT1p, xT23p, cos2, sin2m, ones128, per_core


def kernel(x, W_qkv, W_out, block_mask):
    x = np.asarray(x, dtype=np.float32)
    W_qkv = np.asarray(W_qkv, dtype=np.float32)
    W_out = np.asarray(W_out, dtype=np.float32)
    bm = np.asarray(block_mask).astype(bool)

    plan, biases = _classify_mask(bm)
    key = (plan, biases.shape[0])
    if key not in _prog_cache:
        _prog_cache[key] = _build_program(plan, biases.shape[0])
    nc = _prog_cache[key]

    xT0p, xT1p, xT23p, cos2, sin2m, ones128, per_core = _host_prep(
        x, W_qkv, W_out)
    in_maps = []
    for d in range(N_CORES):
        wqkT, wvT, woutT = per_core[d]
        in_maps.append({
            "xT0p": xT0p, "xT1p": xT1p, "xT23p": xT23p,
            "wqkT": wqkT, "wvT": wvT, "woutT": woutT,
            "cos2": cos2, "sin2m": sin2m,
            "ones128": ones128, "biasT": biases,
        })
    res = bass_utils.run_bass_kernel_spmd(nc, in_maps, list(range(N_CORES)))
    acc = np.zeros((L, D), np.float64)
    for r in res.results:
        acc += r["out"].astype(np.float64)
    return acc.astype(np.float32)[None, :, :]


# revision 27
# speedup vs baseline: 1.0825x; 1.0310x over previous
"""Trainium2 Bass kernel for nn_MultiHeadAttention (fused QKV + RMS-norm +
RoPE + masked softmax attention + out-proj), tensor-parallel over heads
across 8 NeuronCores.

Contract: kernel(**inputs) takes FULL inputs, returns FULL output.
Self-contained: imports only numpy + the concourse framework.

v3: PE-warmup block, 3-way DMA-trigger spread (sync/scalar/gpsimd),
qk/v weight-stream split with paired tile-0 x, pqkfo psum bufs=3,
stats+sums merged into one PSUM bank (rows 0/32 ssq, 64/96 sums) with
batched Ln/Exp and back-to-back 1-row matmuls (PE column-group overlap),
swap-trick rope (6 DVE ops/chain), exp shift removed, out-proj evac
split DVE/ACT/gpsimd with out-DMA triggers on sync.
"""
import numpy as np

import concourse.bacc as bacc
import concourse.mybir as mybir
import concourse.tile as tile
from concourse import bass_utils

# Pin every scalar activation to the one table set that holds all functions
# this kernel uses (Exp, Ln, Square, Copy, Identity) so the table-load
# placement pass never needs a mid-kernel reload. Other sets are presented
# as empty; dict order (= act_func_set_id) is preserved.
_PINNED_SET = "natural_log_exp_and_others"
_orig_get_act_tables = bacc.get_activation_tables


def _pinned_act_tables(arch):
    t = _orig_get_act_tables(arch)
    return {name: (funcs if name == _PINNED_SET else set())
            for name, funcs in t.items()}


bacc.get_activation_tables = _pinned_act_tables

N_HEAD = 16
ROPE_BASE = 100000.0
RMS_EPS = 1e-5
L = 2048
D = 2048
HD = 128          # head dim
N_CORES = 8
HPD = N_HEAD // N_CORES   # heads per device = 2
QT = 512          # q tile (attention + l-tile width)
NQT = L // QT     # 4
NKB = L // 128    # 16 k-blocks
NEG = -1.0e9
SCALE = 1.0 / np.sqrt(HD)

F32 = mybir.dt.float32
F16 = mybir.dt.float16

_prog_cache: dict = {}


def _classify_mask(bm: np.ndarray):
    """Per (q-tile, k-block) classification of the boolean mask.

    plan[qt][kb] = ('full',) | ('skip',) | ('part', bias_idx, bstart, bw,
    zstart, zw): add biases[bias_idx][:, :bw] to score cols
    [bstart:bstart+bw], cols [zstart:zstart+zw] are fully masked. biases is
    [n, 128, 512] fp32 (col-sliced bias subtiles, zero-padded)."""
    plan = []
    uniq = {}
    biases = []
    for qt in range(NQT):
        row = []
        for kb in range(NKB):
            reg = bm[qt * QT:(qt + 1) * QT, kb * 128:(kb + 1) * 128]
            if reg.all():
                row.append(("full",))
                continue
            if not reg.any():
                row.append(("skip",))
                continue
            regT = reg.T  # [128 k, 512 q]
            col_all = regT.all(axis=0)       # fully open columns
            col_none = (~regT).all(axis=0)   # fully masked columns
            mixed = ~(col_all | col_none)
            mix_idx = np.nonzero(mixed)[0]
            none_idx = np.nonzero(col_none)[0]
            # contiguity of ranges (true for causal masks)
            ok = True
            bs = bw = zs = zw = 0
            if mix_idx.size:
                bs, be = int(mix_idx[0]), int(mix_idx[-1]) + 1
                bw = be - bs
                ok &= bool(mixed[bs:be].all())
            if none_idx.size:
                zs, ze = int(none_idx[0]), int(none_idx[-1]) + 1
                zw = ze - zs
                ok &= bool(col_none[zs:ze].all())
                ok &= not (bw and not (ze <= bs or be <= zs))
            if not ok:
                # fallback: full-width bias
                bs, bw, zs, zw = 0, QT, 0, 0
            if bw:
                bias = np.zeros((128, QT), np.float32)
                bias[:, 0:bw] = np.where(regT[:, bs:bs + bw], np.float32(0),
                                         np.float32(NEG))
                key = (bw, bias.tobytes())
                if key not in uniq:
                    uniq[key] = len(biases)
                    biases.append(bias)
                bi = uniq[key]
            else:
                bi = 0
            row.append(("part", bi, bs, bw, zs, zw))
        plan.append(tuple(row))
    if not biases:
        biases.append(np.zeros((128, QT), np.float32))
    return tuple(plan), np.stack(biases)


def _restrict(ent, first):
    """Start column for compute on this block (0 = full width).

    Only left-contiguous fully-masked column runs can be skipped, and the
    first block of an accumulation group must cover all columns."""
    if first or ent[0] != "part":
        return 0
    _, bi, bs, bw, zs, zw = ent
    if zw and zs == 0 and (bw == 0 or bs >= zw):
        return zw
    return 0


def _build_program(plan, n_bias):
    nc = bacc.Bacc("TRN2", target_bir_lowering=False, debug=False)

    # ---- DRAM I/O ----
    # all x streams pair-packed: row-block j holds d-chunks 2j,2j+1 side by
    # side -> 2/4KB DMA lines
    xT0p_d = nc.dram_tensor("xT0p", [D // 2, 2 * QT], F16, kind="ExternalInput")
    xT1p_d = nc.dram_tensor("xT1p", [D // 2, 2 * QT], F16,
                            kind="ExternalInput")
    xT23p_d = nc.dram_tensor("xT23p", [D // 2, 4 * QT], F16,
                             kind="ExternalInput")
    # weights split into qk / v streams (pair-packed like x)
    wqkT_d = nc.dram_tensor("wqkT", [D // 2, 2 * 4 * HD], F16,
                            kind="ExternalInput")
    wvT_d = nc.dram_tensor("wvT", [D // 2, 2 * 2 * HD], F16,
                           kind="ExternalInput")
    woutT_d = nc.dram_tensor("woutT", [2 * HD, D], F16, kind="ExternalInput")
    # rope tables pre-duplicated for the swap-trick rope:
    # cos2 = [cosT; cosT], sin2m = [-sinT; +sinT]  (both [128, L])
    cos2_d = nc.dram_tensor("cos2", [128, L], F16, kind="ExternalInput")
    sin2m_d = nc.dram_tensor("sin2m", [128, L], F16, kind="ExternalInput")
    ones128_d = nc.dram_tensor("ones128", [128, 1], F16, kind="ExternalInput")
    bias_d = nc.dram_tensor("biasT", [n_bias, 128, QT], F32, kind="ExternalInput")
    out_d = nc.dram_tensor("out", [L, D], F16, kind="ExternalOutput")

    Exp = mybir.ActivationFunctionType.Exp
    Ln = mybir.ActivationFunctionType.Ln
    Square = mybir.ActivationFunctionType.Square

    with nc.allow_low_precision(reason="fp16 operands, fp32 accumulation"), \
         tile.TileContext(nc) as tc:
        with (
            tc.tile_pool(name="const", bufs=1) as cpool,
            tc.tile_pool(name="xt", bufs=25) as xtpool,
            tc.tile_pool(name="act", bufs=1) as apool,
            tc.tile_pool(name="wrk", bufs=1) as wpool,
            tc.tile_pool(name="ps", bufs=1, space="PSUM") as ps,
        ):
            # ---- all input DMA triggers on sync, in priority order (the
            # HWDGE in-flight window is global, extra trigger queues only
            # pollute compute queues with throttle waits) ----
            def dma_in(dst, src):
                nc.sync.dma_start(dst, src)

            # ---- PE warm-up: ~12 dummy matmuls (~5us cold) pre-warm the
            # HAM clock gate while the first x/w slices stream in ----
            warm = cpool.tile([128, QT], F16, name="warm", tag="warm")
            nc.vector.memset(warm[:], 0.25)
            for w in range(12):
                wps = ps.tile([128, QT], F32, name=f"warm{w}", tag="sT",
                              bufs=2)
                nc.tensor.matmul(wps[:], warm[:, 0:128], warm[:],
                                 start=True, stop=True)

            # ---- constants + input streams (priority order) ----
            ones128 = cpool.tile([128, 1], F16, name="ones128", tag="ones128")
            dma_in(ones128[:], ones128_d.ap())
            cos2 = cpool.tile([128, L], F16, name="cos2", tag="cos2")
            dma_in(cos2[:], cos2_d.ap())
            sin2m = cpool.tile([128, L], F16, name="sin2m", tag="sin2m")
            dma_in(sin2m[:], sin2m_d.ap())
            epsc = cpool.tile([128, 1], F32, name="epsc", tag="epsc")
            nc.vector.memset(epsc[:], RMS_EPS)

            # tile-0 x (paired) interleaved with qk weight pairs; triggers
            # split across the sync HWDGE and gpsimd SWDGE queues — trigger
            # issue (~0.67us each) is the intro ramp limiter
            wqk_all = cpool.tile([128, 8 * 2 * QT], F16, name="wqk_all",
                                 tag="wqk_all")
            xts0 = []
            for j in range(8):
                e0 = nc.sync if j % 2 == 0 else nc.gpsimd
                e1 = nc.gpsimd if j % 2 == 0 else nc.sync
                xt = xtpool.tile([128, 2 * QT], F16, name="xt0", tag="xt0",
                                 bufs=8)
                e0.dma_start(xt[:], xT0p_d.ap()[j * 128:(j + 1) * 128, :])
                xts0.append(xt)
                e1.dma_start(wqk_all[:, j * 1024:(j + 1) * 1024],
                             wqkT_d.ap()[j * 128:(j + 1) * 128, :])
            # v weights (needed after the first qk chains)
            wv_all = cpool.tile([128, 8 * 512], F16, name="wv_all",
                                tag="wv_all")
            for j in range(8):
                dma_in(wv_all[:, j * 512:(j + 1) * 512],
                       wvT_d.ap()[j * 128:(j + 1) * 128, :])
            # mask biases (needed by qt0 attention)
            btiles = []
            for b in range(n_bias):
                bt = cpool.tile([128, QT], F32, name=f"bias{b}", tag=f"bias{b}")
                dma_in(bt[:], bias_d.ap()[b])
                btiles.append(bt)
            # tile-1 x
            xts1 = []
            for j in range(8):
                xt = xtpool.tile([128, 2 * QT], F16, name="xt1", tag="xt1",
                                 bufs=8)
                dma_in(xt[:], xT1p_d.ap()[j * 128:(j + 1) * 128, :])
                xts1.append(xt)
            # out-proj weights (needed from attention qt1)
            wout_all = cpool.tile([128, 2 * D], F16, name="wout_all",
                                  tag="wout_all")
            for h in range(2):
                dma_in(wout_all[:, h * D:(h + 1) * D],
                       woutT_d.ap()[h * 128:(h + 1) * 128, :])
            # tiles 2+3 x
            xts23 = []
            for j in range(8):
                xt = xtpool.tile([128, 4 * QT], F16, name="xt23", tag="xt23",
                                 bufs=8)
                dma_in(xt[:], xT23p_d.ap()[j * 128:(j + 1) * 128, :])
                xts23.append(xt)

            # ---- persistent activations (fp16) ----
            ktr = [apool.tile([128, L], F16, name=f"ktr{h}", tag=f"ktr{h}")
                   for h in range(HPD)]
            aot = [apool.tile([128, L], F16, name=f"aot{h}", tag=f"aot{h}")
                   for h in range(HPD)]
            vnat = [apool.tile([128, 2 * HD], F16, name=f"vnat{lb}",
                               tag=f"vnat{lb}") for lb in range(16)]

            # ---------- out-projection unit (emitted as attention filler) ----
            # out-DMA triggers lag one unit behind so the sync queue's
            # trigger wait is ~zero by issue time (ob_sb already evacuated)
            pending_dmas = []

            def flush_dmas(alternate=False):
                k = 0
                while pending_dmas:
                    dst, src = pending_dmas.pop(0)
                    eng = nc.gpsimd if (alternate and k % 2) else nc.sync
                    eng.dma_start(dst, src)
                    k += 1

            def make_outproj_units(lt, tail=False):
                units = []
                for idx, (j, jp) in enumerate(
                        (j, jp) for j in range(4) for jp in range(2)):
                    lb = 4 * lt + j
                    # the final units' 256KB out-DMAs are the kernel drain:
                    # split them so pieces ride parallel DMA lanes
                    pieces = 1
                    if tail and idx == 6:
                        pieces = 2
                    elif tail and idx == 7:
                        pieces = 4

                    def unit(lb=lb, jp=jp, pieces=pieces, tail=tail):
                        flush_dmas(alternate=tail)
                        ob_sb = wpool.tile([128, 2 * QT], F16,
                                           name="ob_sb", tag="ob_sb",
                                           bufs=6)
                        for u in range(2):
                            jt = 2 * jp + u
                            fo = ps.tile([128, QT], F32,
                                         name=f"fo{lb}_{jt}",
                                         tag="pqkfo", bufs=3)
                            for h in range(2):
                                nc.tensor.matmul(
                                    fo[:],
                                    aot[h][:, lb * 128:(lb + 1) * 128],
                                    wout_all[:, h * D + jt * QT:
                                             h * D + (jt + 1) * QT],
                                    start=(h == 0), stop=(h == 1),
                                )
                            dst = ob_sb[:, u * QT:(u + 1) * QT]
                            if u == 0:
                                nc.vector.tensor_copy(out=dst, in_=fo[:])
                            else:
                                nc.scalar.copy(dst, fo[:])
                        pw = (2 * QT) // pieces
                        for p in range(pieces):
                            pending_dmas.append((
                                out_d.ap()[lb * 128:(lb + 1) * 128,
                                           jp * 2 * QT + p * pw:
                                           jp * 2 * QT + (p + 1) * pw],
                                ob_sb[:, p * pw:(p + 1) * pw],
                            ))
                    units.append(unit)
                return units

            pending_units = []

            for lt in range(NQT):
                ls = lt * QT
                # ---------- QKV projection for this l-tile ----------
                if lt == 0:
                    xts = [(xts0[i // 2], (i % 2) * QT) for i in range(16)]
                elif lt == 1:
                    xts = [(xts1[i // 2], (i % 2) * QT) for i in range(16)]
                else:
                    xts = [(xts23[i // 2], (i % 2) * 2 * QT + (lt - 2) * QT)
                           for i in range(16)]

                # stats+sums PSUM bank: rows 0/32 = ssq (q-pair then k-pair),
                # rows 64/96 = softmax sums (h0/h1)
                stats = ps.tile([128, QT], F32, name=f"stats{lt}",
                                tag="stats", bufs=1)

                # q/k chains: ob = 0,1 -> q heads 0,1 ; ob = 2,3 -> k heads
                # 0,1; q-pair first so q-rope hides under the k chains
                pp = {}
                qtr_lt = []
                for t in (0, 1):
                    obs = (2 * t, 2 * t + 1)
                    for ob in obs:
                        pp[ob] = ps.tile([128, QT], F32,
                                         name=f"pqk{lt}_{ob}", tag="pqkfo",
                                         bufs=3)
                    if lt == 0:
                        # DMA-inflow-bound: i-outer over the pair
                        for i in range(16):
                            xt, xb = xts[i]
                            for ob in obs:
                                nc.tensor.matmul(
                                    pp[ob][:],
                                    wqk_all[:, (i // 2) * 1024 +
                                            (i % 2) * 512 + ob * 128:
                                            (i // 2) * 1024 +
                                            (i % 2) * 512 + (ob + 1) * 128],
                                    xt[:, xb:xb + QT],
                                    start=(i == 0), stop=(i == 15),
                                )
                    else:
                        for ob in obs:
                            for i in range(16):
                                xt, xb = xts[i]
                                nc.tensor.matmul(
                                    pp[ob][:],
                                    wqk_all[:, (i // 2) * 1024 +
                                            (i % 2) * 512 + ob * 128:
                                            (i // 2) * 1024 +
                                            (i % 2) * 512 + (ob + 1) * 128],
                                    xt[:, xb:xb + QT],
                                    start=(i == 0), stop=(i == 15),
                                )

                    # pair stats: squares on ACT, then two 1-row matmuls
                    # into stats rows 0/32, one batched Ln + Exp over rows
                    # 0..32 (rows 1..31 are garbage, never read)
                    sq = {}
                    for h, ob in enumerate(obs):
                        sqt = wpool.tile([128, QT], F16, name="sq", tag="sq",
                                         bufs=3)
                        nc.scalar.activation(sqt[:], pp[ob][:], Square)
                        sq[ob] = sqt
                    nc.tensor.matmul(stats[0:1, :], ones128[:],
                                     sq[obs[0]][:], start=True, stop=True)
                    nc.tensor.matmul(stats[32:33, :], ones128[:],
                                     sq[obs[1]][:], start=True, stop=True)
                    # s4 = exp(-0.5*ln(ssq/HD + eps)) = 1/sqrt(mean+eps)
                    lg = wpool.tile([33, QT], F32, name="lg", tag="lg",
                                    bufs=2)
                    nc.scalar.activation(lg[:], stats[0:33, :], Ln,
                                         bias=epsc[0:33, :], scale=1.0 / HD)
                    s4b = wpool.tile([33, QT], F16, name="s4", tag="s4",
                                     bufs=2)
                    nc.scalar.activation(s4b[:], lg[:], Exp, scale=-0.5)
                    # h1's scale row copied to a base-0 tile for
                    # partition_broadcast
                    s4h1 = wpool.tile([1, QT], F16, name="s4h1", tag="s4h1",
                                      bufs=2)
                    nc.vector.tensor_copy(out=s4h1[:], in_=s4b[32:33, :])
                    srows = {obs[0]: s4b[0:1, :], obs[1]: s4h1[0:1, :]}

                    # rms-scale + swap-trick rope for the pair:
                    # out[0:64]   = odd*cos - even*sin
                    # out[64:128] = odd*sin + even*cos
                    # via A = scaled*[cos;cos], B = swap(scaled)*[-sin;sin]
                    for h, ob in enumerate(obs):
                        s2 = wpool.tile([128, QT], F16, name="s2", tag="s2",
                                        bufs=3)
                        nc.gpsimd.partition_broadcast(s2[:], srows[ob])
                        # scale reads the raw projection straight from PSUM
                        # (saves a PSUM->SBUF cast per chain)
                        scaled = wpool.tile([128, QT], F16, name="scaled",
                                            tag="scaled", bufs=2)
                        nc.vector.tensor_mul(out=scaled[:], in0=pp[ob][:],
                                             in1=s2[:])
                        sw = wpool.tile([128, QT], F16, name="sw", tag="sw",
                                        bufs=2)
                        nc.vector.tensor_copy(out=sw[0:64, :],
                                              in_=scaled[64:128, :])
                        nc.vector.tensor_copy(out=sw[64:128, :],
                                              in_=scaled[0:64, :])
                        ta = wpool.tile([128, QT], F16, name="ta", tag="ta",
                                        bufs=1)
                        nc.vector.tensor_mul(out=ta[:], in0=scaled[:],
                                             in1=cos2[:, ls:ls + QT])
                        tb = wpool.tile([128, QT], F16, name="tb", tag="tb",
                                        bufs=1)
                        nc.vector.tensor_mul(out=tb[:], in0=sw[:],
                                             in1=sin2m[:, ls:ls + QT])
                        if t == 0:
                            dst = wpool.tile([128, QT], F16, name="qtr",
                                             tag="qtr", bufs=4)
                            nc.vector.tensor_add(out=dst[:], in0=ta[:],
                                                 in1=tb[:])
                            qtr_lt.append(dst)
                        else:
                            nc.vector.tensor_add(
                                out=ktr[h][:, ls:ls + QT],
                                in0=ta[:], in1=tb[:])

                # v projection: natural layout [l, d] per 128-l block
                for j in range(4):
                    lb = 4 * lt + j
                    vp = ps.tile([128, QT], F32, name=f"vp{lb}", tag="pqkfo",
                                 bufs=3)
                    for i in range(16):
                        xt, xb = xts[i]
                        nc.tensor.matmul(
                            vp[:, 0:256],
                            xt[:, xb + j * 128:xb + (j + 1) * 128],
                            wv_all[:, (i // 2) * 512 + (i % 2) * 256:
                                   (i // 2) * 512 + (i % 2 + 1) * 256],
                            start=(i == 0), stop=(i == 15),
                        )
                    nc.vector.tensor_copy(out=vnat[lb][:], in_=vp[:, 0:256])

                # ---------- attention for q-tile qt = lt ----------
                qt = lt
                alive = [kb for kb in range(NKB) if plan[qt][kb][0] != "skip"]
                nblk = len(alive)
                nunits = len(pending_units)
                oT = [ps.tile([128, QT], F32, name=f"oT{qt}_{h}", tag="oT",
                              bufs=2) for h in range(2)]
                emitted = 0
                # both heads' sums matmuls lag one block so the pair
                # [sums_h0(n-1), sums_h1(n-1)] is issued adjacently with both
                # pt inputs long-ready -> the two 1-row matmuls overlap in
                # distinct PE column groups (rows 0/64)
                lag1 = None  # (pts, zr, start, stop) for the prev block
                for n, kb in enumerate(alive):
                    ent = plan[qt][kb]
                    zr = _restrict(ent, n == 0)
                    sts = []
                    for h in range(2):
                        st = ps.tile([128, QT], F32, name=f"sT{qt}_{h}_{n}",
                                     tag="sT", bufs=2)
                        nc.tensor.matmul(
                            st[:, zr:],
                            ktr[h][:, kb * 128:(kb + 1) * 128],
                            qtr_lt[h][:, zr:],
                            start=True, stop=True,
                        )
                        sts.append(st)
                    if ent[0] == "part":
                        _, bi, bs, bw, zs, zw = ent
                        for h in range(2):
                            if bw:
                                nc.vector.tensor_add(
                                    out=sts[h][:, bs:bs + bw],
                                    in0=sts[h][:, bs:bs + bw],
                                    in1=btiles[bi][:, 0:bw])
                            if zw and zr == 0:
                                nc.vector.tensor_scalar_add(
                                    out=sts[h][:, zs:zs + zw],
                                    in0=sts[h][:, zs:zs + zw],
                                    scalar1=NEG)
                    pts = []
                    for h in range(2):
                        pt = wpool.tile([128, QT], F16, name="pt", tag="pt",
                                        bufs=6)
                        nc.scalar.activation(pt[:, zr:], sts[h][:, zr:], Exp,
                                             scale=SCALE)
                        pts.append(pt)
                    if lag1 is not None:
                        lpts, lzr, lstart, lstop = lag1
                        nc.tensor.matmul(
                            stats[0:1, lzr:], ones128[:], lpts[0][:, lzr:],
                            start=lstart, stop=lstop)
                        nc.tensor.matmul(
                            stats[64:65, lzr:], ones128[:], lpts[1][:, lzr:],
                            start=lstart, stop=lstop)
                    lag1 = (pts, zr, n == 0, n == nblk - 1)
                    for h in range(2):
                        nc.tensor.matmul(
                            oT[h][:, zr:],
                            vnat[kb][:, h * HD:(h + 1) * HD],
                            pts[h][:, zr:],
                            start=(n == 0), stop=(n == nblk - 1),
                        )
                    # interleave out-proj filler from the previous l-tile;
                    # on the last l-tile hold 2 units back to cover the final
                    # normalize latency after the block loop
                    cap = nunits - 2 if lt == NQT - 1 else nunits
                    want = min((nunits * (n + 1)) // nblk, cap)
                    while emitted < want:
                        pending_units[emitted]()
                        emitted += 1
                while emitted < nunits:
                    pending_units[emitted]()
                    emitted += 1
                # flush the lagged sums for the final block
                if lag1 is not None:
                    lpts, lzr, lstart, lstop = lag1
                    nc.tensor.matmul(
                        stats[0:1, lzr:], ones128[:], lpts[0][:, lzr:],
                        start=lstart, stop=lstop)
                    nc.tensor.matmul(
                        stats[64:65, lzr:], ones128[:], lpts[1][:, lzr:],
                        start=lstart, stop=lstop)

                # normalize: aot[h][:, qt] = oT * (1/sums) bcast over
                # partitions; per-head recips so h0's broadcast starts while
                # h1's recip runs (shortens the critical path on the last
                # tile, whose out-proj waits on this)
                # batched reciprocal over stats rows 0..64 (base-0 input —
                # DVE table ops misread base-64 APs), then h1's row copied
                # down for its broadcast
                rinvb = wpool.tile([65, QT], F32, name="rinvb", tag="rinvb",
                                   bufs=2)
                nc.vector.reciprocal_approx_fast(out=rinvb[:],
                                                 in_=stats[0:65, :])
                r1 = wpool.tile([1, QT], F32, name="r1", tag="r1", bufs=2)
                nc.vector.tensor_copy(out=r1[:], in_=rinvb[64:65, :])
                for h in range(2):
                    rb = wpool.tile([128, QT], F32, name="rb", tag="rb",
                                    bufs=2)
                    nc.gpsimd.partition_broadcast(
                        rb[:], rinvb[0:1, :] if h == 0 else r1[0:1, :])
                    nc.vector.tensor_mul(
                        out=aot[h][:, qt * QT:(qt + 1) * QT],
                        in0=oT[h][:], in1=rb[:],
                    )

                pending_units = make_outproj_units(lt, tail=(lt == NQT - 1))

            # last l-tile's out-projection; drain DMAs on two queues
            for unit in pending_units:
                unit()
            flush_dmas(alternate=True)

    nc.finalize()
    return nc


def _rope_perm(h):
    """Row order within one head's 128 q/k features: odd indices then even."""
    base = h * HD
    return np.concatenate([np.arange(1, HD, 2), np.arange(0, HD, 2)]) + base


def _pack16(a):
    """[2048, w] -> [1024, 2w]: row-block j holds d-chunks 2j, 2j+1 side by
    side (bigger DMA lines)."""
    w = a.shape[1]
    r = a.reshape(16, 128, w)
    return np.ascontiguousarray(
        np.concatenate([r[0::2], r[1::2]], axis=2).reshape(1024, 2 * w))


def _host_prep(x, W_qkv, W_out):
    xT = np.ascontiguousarray(x[0].T.astype(np.float16))
    xT0p = _pack16(xT[:, 0:QT])
    xT1p = _pack16(xT[:, QT:2 * QT])
    xT23p = _pack16(xT[:, 2 * QT:4 * QT])
    inv_freq = 1.0 / (ROPE_BASE ** (np.arange(0, HD, 2, dtype=np.float64) / HD))
    ang = np.arange(L, dtype=np.float64)[:, None] * inv_freq[None, :]
    cosT = np.cos(ang).T
    sinT = np.sin(ang).T
    cos2 = np.ascontiguousarray(
        np.vstack([cosT, cosT]).astype(np.float16))
    sin2m = np.ascontiguousarray(
        np.vstack([-sinT, sinT]).astype(np.float16))
    ones128 = np.ones((128, 1), np.float16)

    per_core = []
    for d in range(N_CORES):
        h0 = HPD * d
        rows_q = np.concatenate([_rope_perm(h0), _rope_perm(h0 + 1)])
        rows = np.concatenate(
            [rows_q, D + rows_q,
             2 * D + np.arange(h0 * HD, (h0 + 2) * HD)]
        )
        wl = W_qkv[rows, :]                                     # [768, 2048]
        wqkvT = wl.T.astype(np.float16)                         # [2048, 768]
        wqkT = _pack16(wqkvT[:, 0:512])                         # [1024, 1024]
        wvT = _pack16(wqkvT[:, 512:768])                        # [1024, 512]
        woutT = np.ascontiguousarray(
            W_out[:, h0 * HD:(h0 + 2) * HD].T.astype(np.float16)
        )                                                       # [256, 2048]
        per_core.append((wqkT, wvT, woutT))
    return xT0p, xT1p, xT23p, cos2, sin2m, ones128, per_core


def kernel(x, W_qkv, W_out, block_mask):
    x = np.asarray(x, dtype=np.float32)
    W_qkv = np.asarray(W_qkv, dtype=np.float32)
    W_out = np.asarray(W_out, dtype=np.float32)
    bm = np.asarray(block_mask).astype(bool)

    plan, biases = _classify_mask(bm)
    key = (plan, biases.shape[0])
    if key not in _prog_cache:
        _prog_cache[key] = _build_program(plan, biases.shape[0])
    nc = _prog_cache[key]

    xT0p, xT1p, xT23p, cos2, sin2m, ones128, per_core = _host_prep(
        x, W_qkv, W_out)
    in_maps = []
    for d in range(N_CORES):
        wqkT, wvT, woutT = per_core[d]
        in_maps.append({
            "xT0p": xT0p, "xT1p": xT1p, "xT23p": xT23p,
            "wqkT": wqkT, "wvT": wvT, "woutT": woutT,
            "cos2": cos2, "sin2m": sin2m,
            "ones128": ones128, "biasT": biases,
        })
    res = bass_utils.run_bass_kernel_spmd(nc, in_maps, list(range(N_CORES)))
    acc = np.zeros((L, D), np.float64)
    for r in res.results:
        acc += r["out"].astype(np.float64)
    return acc.astype(np.float32)[None, :, :]
